# revision 16
# baseline (speedup 1.0000x reference)
"""Causal MHA (B=4, S=2048, D=1024, H=16) on 8 TRN2 NeuronCores.

Sharding: tensor-parallel over heads. Core c owns heads {2c, 2c+1} = feature
slice [128c : 128c+128].  Each core:
  1. projects Q^T, K^T (feature-major [128, S] per batch) and V (token-major)
     from a replicated bf16 x^T,
  2. applies RoPE to Q^T/K^T on the vector engine (pair-swap stream_shuffle +
     cos/sin tables),
  3. computes causal attention per (batch, head): S^T = K^T.T @ Q^T blocks
     (two heads row-tiled in the PE array), additive -400 mask via an
     identity matmul on diagonal blocks, exp on the scalar engine,
     P~ @ V with a ones-block in the V stationary operand so the softmax
     denominator lands in PSUM rows 64:128 for free,
  4. output-projects its feature slice against full Wo (partial sums),
  5. host sums the 8 partial y^T and unshards.
All matmuls bf16 with fp32 PSUM accumulation; softmax uses exp-without-max
(scores are O(5) for this regime) in fp32.
"""
import numpy as np
import ml_dtypes

import concourse.bass as bass
import concourse.bacc as bacc
import concourse.tile as tile
import concourse.mybir as mybir

F32 = mybir.dt.float32
BF16 = mybir.dt.bfloat16

B, S, D, H, DK = 4, 2048, 1024, 16, 64
NC = 8                    # cores
DSL = D // NC             # per-core feature slice = 128
QT = 512                  # query tile (free dim of S^T blocks)
KCH = 128                 # key chunk (partition dim of S^T blocks)
THETA = 10000.0
MASK_NEG = -400.0         # additive mask pre-scale; *0.125 = -50 at exp input

_PAIR_SWAP = []
for _i in range(16):
    _PAIR_SWAP += [2 * _i + 1, 2 * _i]


def build_bass(b_count=B, s_len=S, debug_taps=False, repeat=1):
    nqt = s_len // QT
    nkc = s_len // KCH
    bs = b_count * s_len

    nc = bacc.Bacc()
    xT = nc.declare_dram_parameter("xT", [D, bs], BF16, isOutput=False)
    wq = nc.declare_dram_parameter("wq", [D, DSL], BF16, isOutput=False)
    wk = nc.declare_dram_parameter("wk", [D, DSL], BF16, isOutput=False)
    wv = nc.declare_dram_parameter("wv", [D, DSL], BF16, isOutput=False)
    wo = nc.declare_dram_parameter("wo", [DSL, D], BF16, isOutput=False)
    ropeC = nc.declare_dram_parameter("ropeC", [128, s_len], F32, isOutput=False)
    ropeS = nc.declare_dram_parameter("ropeS", [128, s_len], F32, isOutput=False)
    maskb = nc.declare_dram_parameter("maskb", [128, 4 * QT], BF16, isOutput=False)
    ident = nc.declare_dram_parameter("ident", [128, 128], BF16, isOutput=False)
    yT = nc.declare_dram_parameter("yT", [D, bs], BF16, isOutput=True)
    taps = {}
    if debug_taps:
        taps["qt0"] = nc.declare_dram_parameter("dbg_qt0", [128, s_len], BF16, isOutput=True)
        taps["kt0"] = nc.declare_dram_parameter("dbg_kt0", [128, s_len], BF16, isOutput=True)
        taps["vp0"] = nc.declare_dram_parameter("dbg_vp0", [128, (s_len // KCH) * 2 * 128], BF16, isOutput=True)
        taps["ot0"] = nc.declare_dram_parameter("dbg_ot0", [128, s_len], BF16, isOutput=True)
        taps["pt0"] = nc.declare_dram_parameter("dbg_pt0", [128, 2 * QT], BF16, isOutput=True)
        taps["op0"] = nc.declare_dram_parameter("dbg_op0", [128, QT], F32, isOutput=True)
        taps["op1"] = nc.declare_dram_parameter("dbg_op1", [128, QT], F32, isOutput=True)
        taps["rec0"] = nc.declare_dram_parameter("dbg_rec0", [64, QT], F32, isOutput=True)

    with tile.TileContext(nc) as tc:
        from contextlib import ExitStack
        with ExitStack() as ctx:
            consts = ctx.enter_context(tc.tile_pool(name="consts", bufs=1))
            xpool = ctx.enter_context(tc.tile_pool(name="xpool", bufs=48))
            qkpool = ctx.enter_context(tc.tile_pool(name="qkpool", bufs=2))
            vpool = ctx.enter_context(tc.tile_pool(name="vpool", bufs=2))
            otpool = ctx.enter_context(tc.tile_pool(name="otpool", bufs=2))
            rtmp = ctx.enter_context(tc.tile_pool(name="rtmp", bufs=3))
            ppool = ctx.enter_context(tc.tile_pool(name="ppool", bufs=3))
            rcpool = ctx.enter_context(tc.tile_pool(name="rcpool", bufs=4))
            ypool = ctx.enter_context(tc.tile_pool(name="ypool", bufs=4))
            psA = ctx.enter_context(tc.tile_pool(name="psA", bufs=2, space="PSUM"))
            psP = ctx.enter_context(tc.tile_pool(name="psP", bufs=2, space="PSUM"))
            psO = ctx.enter_context(tc.tile_pool(name="psO", bufs=2, space="PSUM"))

            # ---- constants ----
            wq_sb = consts.tile([128, 8, DSL], BF16)
            wk_sb = consts.tile([128, 8, DSL], BF16)
            wv_sb = consts.tile([128, 8, DSL], BF16)
            nc.sync.dma_start(wq_sb[:], wq.rearrange("(c p) m -> p c m", p=128))
            nc.sync.dma_start(wk_sb[:], wk.rearrange("(c p) m -> p c m", p=128))
            nc.sync.dma_start(wv_sb[:], wv.rearrange("(c p) m -> p c m", p=128))
            wo_sb = consts.tile([128, 8, 128], BF16)
            nc.sync.dma_start(wo_sb[:], wo.rearrange("p (c m) -> p c m", m=128))
            rc_sb = consts.tile([128, s_len], F32)
            rs_sb = consts.tile([128, s_len], F32)
            nc.sync.dma_start(rc_sb[:], ropeC[:])
            nc.sync.dma_start(rs_sb[:], ropeS[:])
            mb_sb = consts.tile([128, 4, QT], BF16)
            nc.sync.dma_start(mb_sb[:], maskb.rearrange("p (r m) -> p r m", m=QT))
            id_sb = consts.tile([128, 128], BF16)
            nc.sync.dma_start(id_sb[:], ident[:])
            rep_ctx = tc.For_i(0, repeat, 1) if repeat > 1 else None
            if rep_ctx is not None:
                rep_ctx.__enter__()
            touch = consts.tile([1, 16], F32)
            for _i, _t in enumerate((wq_sb, wk_sb, wv_sb, wo_sb, mb_sb, id_sb)):
                nc.vector.tensor_copy(touch[0:1, _i:_i + 1], _t[0:1, 0:1, 0:1]
                                      if len(_t.shape) == 3 else _t[0:1, 0:1])
            nc.vector.tensor_copy(touch[0:1, 6:7], rc_sb[0:1, 0:1])
            nc.vector.tensor_copy(touch[0:1, 7:8], rs_sb[0:1, 0:1])

            for b in range(b_count):
                base = b * s_len
                xt = {}     # (qt, ch) -> sbuf tile [128, 512]

                def xtile(qt, ch):
                    if (qt, ch) not in xt:
                        t = xpool.tile([128, QT], BF16, tag="x")
                        nc.sync.dma_start(
                            t[:], xT[128 * ch:128 * (ch + 1),
                                     base + QT * qt: base + QT * (qt + 1)])
                        xt[(qt, ch)] = t
                    return xt[(qt, ch)]

                # ---- Q^T / K^T projections + rope ----
                qt_b = qkpool.tile([128, s_len], BF16, tag="qt")
                kt_b = qkpool.tile([128, s_len], BF16, tag="kt")
                for name, w_sb, dst in (("q", wq_sb, qt_b), ("k", wk_sb, kt_b)):
                    for qt in range(nqt):
                        ps = psA.tile([128, QT], F32, tag="a")
                        for ch in range(8):
                            nc.tensor.matmul(ps[:], w_sb[:, ch, :], xtile(qt, ch)[:],
                                             start=(ch == 0), stop=(ch == 7))
                        cs = slice(QT * qt, QT * (qt + 1))
                        psb = rtmp.tile([128, QT], F32, tag="pb")
                        nc.scalar.copy(psb[:], ps[:])
                        qsw = rtmp.tile([128, QT], F32, tag="sw")
                        m1 = rtmp.tile([128, QT], F32, tag="m1")
                        m2 = rtmp.tile([128, QT], F32, tag="m2")
                        nc.vector.stream_shuffle(qsw[:], ps[:], _PAIR_SWAP)
                        nc.gpsimd.tensor_mul(m1[:], psb[:], rc_sb[:, cs])
                        nc.gpsimd.tensor_mul(m2[:], qsw[:], rs_sb[:, cs])
                        nc.gpsimd.tensor_add(dst[:, cs], m1[:], m2[:])

                # ---- V projection: orientation B (V^T) + PE transpose ----
                vp = vpool.tile([128, nkc, 2, 128], BF16, tag="v")
                nc.vector.memset(vp[:, :, :, 64:128], 1.0)
                for qt in range(nqt):
                    vps = psA.tile([128, QT], F32, tag="a")
                    for ch in range(8):
                        nc.tensor.matmul(vps[:], wv_sb[:, ch, :], xtile(qt, ch)[:],
                                         start=(ch == 0), stop=(ch == 7))
                    vtb = rtmp.tile([128, QT], BF16, tag="vb")
                    nc.vector.tensor_copy(vtb[:], vps[:])
                    vtp = psO.tile([128, QT], BF16, tag="o")
                    for j in range(4):
                        nc.tensor.transpose(vtp[:, 128 * j:128 * (j + 1)],
                                            vtb[:, 128 * j:128 * (j + 1)], id_sb[:])
                    for j in range(4):
                        kc = 4 * qt + j
                        nc.vector.tensor_copy(
                            vp[:, kc, :, 0:64],
                            vtp[:, 128 * j:128 * (j + 1)].rearrange(
                                "p (h m) -> p h m", h=2))

                if debug_taps and b == 0:
                    nc.sync.dma_start(taps["qt0"][:], qt_b[:])
                    nc.sync.dma_start(taps["kt0"][:], kt_b[:])
                    nc.sync.dma_start(taps["vp0"][:], vp[:].rearrange("p a b m -> p (a b m)"))
                # ---- attention ----
                ot_b = otpool.tile([128, s_len], BF16, tag="ot")
                for qt in range(nqt):
                    qs = slice(QT * qt, QT * (qt + 1))
                    nkc_q = 4 * (qt + 1)
                    op0 = psO.tile([128, QT], F32, tag="o")
                    op1 = psO.tile([128, QT], F32, tag="o")
                    op = [op0, op1]
                    for kc in range(nkc_q):
                        pair = psP.tile([128, 2 * QT], F32, tag="p")
                        ks = slice(KCH * kc, KCH * (kc + 1))
                        diag = kc >= 4 * qt
                        nc.tensor.matmul(pair[:, 0:QT], kt_b[0:64, ks],
                                         qt_b[0:64, qs], start=True,
                                         stop=not diag, tile_position=(0, 0))
                        nc.tensor.matmul(pair[:, QT:2 * QT], kt_b[64:128, ks],
                                         qt_b[64:128, qs], start=True,
                                         stop=not diag, tile_position=(64, 0))
                        if diag:
                            r = kc - 4 * qt
                            nc.tensor.matmul(pair[:, 0:QT], id_sb[:],
                                             mb_sb[:, r, :], start=False, stop=True)
                            nc.tensor.matmul(pair[:, QT:2 * QT], id_sb[:],
                                             mb_sb[:, r, :], start=False, stop=True)
                        pt = ppool.tile([128, 2 * QT], BF16, tag="pt")
                        nc.scalar.activation(pt[:], pair[:],
                                             mybir.ActivationFunctionType.Exp,
                                             scale=0.125)
                        if debug_taps and b == 0 and qt == 0 and kc == 0:
                            nc.sync.dma_start(taps["pt0"][:], pt[:])
                        for h in range(2):
                            nc.tensor.matmul(op[h][:], vp[:, kc, h, :],
                                             pt[:, QT * h:QT * (h + 1)],
                                             start=(kc == 0), stop=(kc == nkc_q - 1))
                    for h in range(2):
                        rstg = rcpool.tile([64, QT], F32, tag="rs")
                        nc.scalar.copy(rstg[:], op[h][64:128, :])
                        ostg = rcpool.tile([64, QT], F32, tag="os")
                        nc.scalar.copy(ostg[:], op[h][0:64, :])
                        rec = rcpool.tile([64, QT], F32, tag="rc")
                        nc.vector.reciprocal_approx_fast(rec[:], rstg[:])
                        if debug_taps and b == 0 and qt == 0:
                            opd = rcpool.tile([128, QT], F32, tag="opd")
                            nc.vector.tensor_copy(opd[:], op[h][:])
                            nc.sync.dma_start(taps[f"op{h}"][:], opd[:])
                            if h == 0:
                                nc.sync.dma_start(taps["rec0"][:], rec[:])
                        nc.vector.tensor_mul(ot_b[64 * h:64 * h + 64, qs],
                                             ostg[:], rec[:])

                if debug_taps and b == 0:
                    nc.sync.dma_start(taps["ot0"][:], ot_b[:])
                # ---- output projection (partial, vs full Wo) ----
                for ec in range(8):
                    for tt in range(nqt):
                        ts_ = slice(QT * tt, QT * (tt + 1))
                        yps = psA.tile([128, QT], F32, tag="a")
                        nc.tensor.matmul(yps[:], wo_sb[:, ec, :], ot_b[:, ts_],
                                         start=True, stop=True)
                        nc.vector.tensor_copy(touch[0:1, 8:9], yps[0:1, 0:1])
                        ysb = ypool.tile([128, QT], BF16, tag="y")
                        nc.vector.tensor_copy(ysb[:], yps[:])
                        nc.sync.dma_start(
                            yT[128 * ec:128 * (ec + 1),
                               base + QT * tt: base + QT * (tt + 1)], ysb[:])
            if rep_ctx is not None:
                rep_ctx.__exit__(None, None, None)
    nc.finalize()
    return nc


def host_inputs(x, token_pos, Wq, Wk, Wv, Wo, b_count=B, s_len=S):
    """Build the 8 per-core input maps (host-side prep: transpose/cast/tables)."""
    bs = b_count * s_len
    bf = ml_dtypes.bfloat16
    xTf = np.ascontiguousarray(
        np.asarray(x, np.float32).reshape(bs, D).T).astype(bf)

    tp = np.asarray(token_pos, np.float64)[:s_len]
    freqs = 1.0 / THETA ** (np.arange(0, DK, 2, dtype=np.float64) / DK)  # [32]
    ang = tp[:, None] * freqs[None, :]                  # [S, 32]
    cos = np.cos(ang).T                                 # [32, S]
    sin = np.sin(ang).T
    # d-major rows for one head (64): row d -> table d//2; sign s*(-1 if even)
    C64 = np.repeat(cos, 2, axis=0).astype(np.float32)  # [64, S]
    S64 = np.repeat(sin, 2, axis=0).astype(np.float32)
    sgn = np.where(np.arange(64) % 2 == 0, -1.0, 1.0)[:, None].astype(np.float32)
    ropeC = np.ascontiguousarray(np.tile(C64, (2, 1)))            # [128, S]
    ropeS = np.ascontiguousarray(np.tile(S64 * sgn, (2, 1)))      # [128, S]

    k_idx = np.arange(128)[:, None]
    j_idx = np.arange(QT)[None, :]
    masks = []
    for r in range(4):
        masks.append(np.where(j_idx >= 128 * r + k_idx, 0.0, MASK_NEG))
    maskb = np.concatenate(masks, axis=1).astype(bf)     # [128, 2048]
    identity = np.eye(128, dtype=np.float32).astype(bf)

    WqT = np.asarray(Wq, np.float32).T.astype(bf)        # [din, dout]
    WkT = np.asarray(Wk, np.float32).T.astype(bf)
    WvT = np.asarray(Wv, np.float32).T.astype(bf)
    WoT = np.asarray(Wo, np.float32).T.astype(bf)        # [d, e]

    in_maps = []
    for c in range(NC):
        sl = slice(DSL * c, DSL * (c + 1))
        in_maps.append({
            "xT": xTf,
            "wq": np.ascontiguousarray(WqT[:, sl]),
            "wk": np.ascontiguousarray(WkT[:, sl]),
            "wv": np.ascontiguousarray(WvT[:, sl]),
            "wo": np.ascontiguousarray(WoT[sl, :]),
            "ropeC": ropeC,
            "ropeS": ropeS,
            "maskb": maskb,
            "ident": identity,
        })
    return in_maps


def unshard(results, b_count=B, s_len=S):
    bs = b_count * s_len
    acc = np.zeros((D, bs), np.float32)
    for r in results:
        acc += np.asarray(r["yT"]).astype(np.float32)
    return np.ascontiguousarray(acc.T).reshape(b_count, s_len, D)


def kernel(x, token_pos, Wq, Wk, Wv, Wo):
    from concourse.bass_utils import run_bass_kernel_spmd
    nc = build_bass()
    in_maps = host_inputs(x, token_pos, Wq, Wk, Wv, Wo)
    res = run_bass_kernel_spmd(nc, in_maps, list(range(NC)))
    return unshard(res.results).astype(np.float32)


# revision 25
# speedup vs baseline: 1.8097x; 1.8097x over previous
"""Causal MHA (B=4, S=2048, D=1024, H=16) on 8 TRN2 NeuronCores.

Sharding: tensor-parallel over heads. Core c owns heads {2c, 2c+1} = feature
slice [128c : 128c+128].  Each core:
  1. projects Q^T, K^T (feature-major [128, S] per batch) and V (token-major)
     from a replicated bf16 x^T,
  2. applies RoPE to Q^T/K^T on the vector engine (pair-swap stream_shuffle +
     cos/sin tables),
  3. computes causal attention per (batch, head): S^T = K^T.T @ Q^T blocks
     (two heads row-tiled in the PE array), additive -400 mask via an
     identity matmul on diagonal blocks, exp on the scalar engine,
     P~ @ V with a ones-block in the V stationary operand so the softmax
     denominator lands in PSUM rows 64:128 for free,
  4. output-projects its feature slice against full Wo (partial sums),
  5. host sums the 8 partial y^T and unshards.
All matmuls bf16 with fp32 PSUM accumulation; softmax uses exp-without-max
(scores are O(5) for this regime) in fp32.
"""
import numpy as np
import ml_dtypes

import concourse.bass as bass
import concourse.bacc as bacc
import concourse.tile as tile
import concourse.mybir as mybir

F32 = mybir.dt.float32
BF16 = mybir.dt.bfloat16

B, S, D, H, DK = 4, 2048, 1024, 16, 64
NC = 8                    # cores
DSL = D // NC             # per-core feature slice = 128
QT = 512                  # query tile (free dim of S^T blocks)
KCH = 128                 # key chunk (partition dim of S^T blocks)
THETA = 10000.0
MASK_NEG = -400.0         # additive mask pre-scale; *0.125 = -50 at exp input

_PAIR_SWAP = []
for _i in range(16):
    _PAIR_SWAP += [2 * _i + 1, 2 * _i]


def build_bass(b_count=B, s_len=S, debug_taps=False, repeat=1):
    nqt = s_len // QT
    nkc = s_len // KCH
    bs = b_count * s_len

    nc = bacc.Bacc()
    xT = nc.declare_dram_parameter("xT", [D, bs], BF16, isOutput=False)
    wq = nc.declare_dram_parameter("wq", [D, DSL], BF16, isOutput=False)
    wk = nc.declare_dram_parameter("wk", [D, DSL], BF16, isOutput=False)
    wv = nc.declare_dram_parameter("wv", [D, DSL], BF16, isOutput=False)
    wo = nc.declare_dram_parameter("wo", [DSL, D], BF16, isOutput=False)
    ropeC = nc.declare_dram_parameter("ropeC", [128, s_len], F32, isOutput=False)
    ropeS = nc.declare_dram_parameter("ropeS", [128, s_len], F32, isOutput=False)
    maskb = nc.declare_dram_parameter("maskb", [128, 4 * QT], BF16, isOutput=False)
    ident = nc.declare_dram_parameter("ident", [128, 128], BF16, isOutput=False)
    yT = nc.declare_dram_parameter("yT", [D, bs], BF16, isOutput=True)
    taps = {}
    if debug_taps:
        taps["qt0"] = nc.declare_dram_parameter("dbg_qt0", [128, s_len], BF16, isOutput=True)
        taps["kt0"] = nc.declare_dram_parameter("dbg_kt0", [128, s_len], BF16, isOutput=True)
        taps["vp0"] = nc.declare_dram_parameter("dbg_vp0", [128, (s_len // KCH) * 2 * 128], BF16, isOutput=True)
        taps["ot0"] = nc.declare_dram_parameter("dbg_ot0", [128, s_len], BF16, isOutput=True)
        taps["pt0"] = nc.declare_dram_parameter("dbg_pt0", [128, 2 * QT], BF16, isOutput=True)
        taps["op0"] = nc.declare_dram_parameter("dbg_op0", [128, QT], F32, isOutput=True)
        taps["op1"] = nc.declare_dram_parameter("dbg_op1", [128, QT], F32, isOutput=True)
        taps["rec0"] = nc.declare_dram_parameter("dbg_rec0", [64, QT], F32, isOutput=True)

    with tile.TileContext(nc) as tc:
        from contextlib import ExitStack
        with ExitStack() as ctx:
            consts = ctx.enter_context(tc.tile_pool(name="consts", bufs=1))
            xpool = ctx.enter_context(tc.tile_pool(name="xpool", bufs=48))
            qkpool = ctx.enter_context(tc.tile_pool(name="qkpool", bufs=2))
            vpool = ctx.enter_context(tc.tile_pool(name="vpool", bufs=2))
            otpool = ctx.enter_context(tc.tile_pool(name="otpool", bufs=2))
            rtmp = ctx.enter_context(tc.tile_pool(name="rtmp", bufs=3))
            ppool = ctx.enter_context(tc.tile_pool(name="ppool", bufs=4))
            rcpool = ctx.enter_context(tc.tile_pool(name="rcpool", bufs=4))
            ypool = ctx.enter_context(tc.tile_pool(name="ypool", bufs=4))
            psA = ctx.enter_context(tc.tile_pool(name="psA", bufs=2, space="PSUM"))
            psP = ctx.enter_context(tc.tile_pool(name="psP", bufs=2, space="PSUM"))
            psO = ctx.enter_context(tc.tile_pool(name="psO", bufs=2, space="PSUM"))

            # ---- constants ----
            wq_sb = consts.tile([128, 8, DSL], BF16)
            wk_sb = consts.tile([128, 8, DSL], BF16)
            wv_sb = consts.tile([128, 8, DSL], BF16)
            nc.sync.dma_start(wq_sb[:], wq.rearrange("(c p) m -> p c m", p=128))
            nc.sync.dma_start(wk_sb[:], wk.rearrange("(c p) m -> p c m", p=128))
            nc.sync.dma_start(wv_sb[:], wv.rearrange("(c p) m -> p c m", p=128))
            wo_sb = consts.tile([128, 8, 128], BF16)
            nc.sync.dma_start(wo_sb[:], wo.rearrange("p (c m) -> p c m", m=128))
            rc_sb = consts.tile([128, s_len], F32)
            rs_sb = consts.tile([128, s_len], F32)
            nc.sync.dma_start(rc_sb[:], ropeC[:])
            nc.sync.dma_start(rs_sb[:], ropeS[:])
            mb_sb = consts.tile([128, 4, QT], BF16)
            nc.sync.dma_start(mb_sb[:], maskb.rearrange("p (r m) -> p r m", m=QT))
            id_sb = consts.tile([128, 128], BF16)
            nc.sync.dma_start(id_sb[:], ident[:])
            rep_ctx = tc.For_i(0, repeat, 1) if repeat > 1 else None
            if rep_ctx is not None:
                rep_ctx.__enter__()

            for b in range(b_count):
                base = b * s_len
                xt = {}     # (qt, ch) -> sbuf tile [128, 512]

                def xtile(qt, ch):
                    if (qt, ch) not in xt:
                        t = xpool.tile([128, QT], BF16, tag="x")
                        nc.sync.dma_start(
                            t[:], xT[128 * ch:128 * (ch + 1),
                                     base + QT * qt: base + QT * (qt + 1)])
                        xt[(qt, ch)] = t
                    return xt[(qt, ch)]

                # ---- Q^T / K^T projections + rope ----
                qt_b = qkpool.tile([128, s_len], BF16, tag="qt")
                kt_b = qkpool.tile([128, s_len], BF16, tag="kt")
                for name, w_sb, dst in (("q", wq_sb, qt_b), ("k", wk_sb, kt_b)):
                    for qt in range(nqt):
                        ps = psA.tile([128, QT], F32, tag="a")
                        for ch in range(8):
                            nc.tensor.matmul(ps[:], w_sb[:, ch, :], xtile(qt, ch)[:],
                                             start=(ch == 0), stop=(ch == 7))
                        cs = slice(QT * qt, QT * (qt + 1))
                        psb = rtmp.tile([128, QT], F32, tag="pb")
                        nc.scalar.copy(psb[:], ps[:])
                        qsw = rtmp.tile([128, QT], F32, tag="sw")
                        m1 = rtmp.tile([128, QT], F32, tag="m1")
                        m2 = rtmp.tile([128, QT], F32, tag="m2")
                        nc.vector.stream_shuffle(qsw[:], ps[:], _PAIR_SWAP)
                        nc.gpsimd.tensor_mul(m1[:], psb[:], rc_sb[:, cs])
                        nc.gpsimd.tensor_mul(m2[:], qsw[:], rs_sb[:, cs])
                        nc.gpsimd.tensor_add(dst[:, cs], m1[:], m2[:])

                # ---- V projection: orientation B (V^T) + PE transpose ----
                vp = vpool.tile([128, nkc, 2, 128], BF16, tag="v")
                nc.vector.memset(vp[:, :, :, 0:64], 1.0)
                for qt in range(nqt):
                    vps = psA.tile([128, QT], F32, tag="a")
                    for ch in range(8):
                        nc.tensor.matmul(vps[:], wv_sb[:, ch, :], xtile(qt, ch)[:],
                                         start=(ch == 0), stop=(ch == 7))
                    vtb = rtmp.tile([128, QT], BF16, tag="vb")
                    nc.vector.tensor_copy(vtb[:], vps[:])
                    vtp = psO.tile([128, QT], BF16, tag="o")
                    for j in range(4):
                        nc.tensor.transpose(vtp[:, 128 * j:128 * (j + 1)],
                                            vtb[:, 128 * j:128 * (j + 1)], id_sb[:])
                    for j in range(4):
                        kc = 4 * qt + j
                        nc.vector.tensor_copy(
                            vp[:, kc, :, 64:128],
                            vtp[:, 128 * j:128 * (j + 1)].rearrange(
                                "p (h m) -> p h m", h=2))

                if debug_taps and b == 0:
                    nc.sync.dma_start(taps["qt0"][:], qt_b[:])
                    nc.sync.dma_start(taps["kt0"][:], kt_b[:])
                    nc.sync.dma_start(taps["vp0"][:], vp[:].rearrange("p a b m -> p (a b m)"))
                # ---- attention ----
                ot_b = otpool.tile([128, s_len], BF16, tag="ot")
                for qt in range(nqt):
                    qs = slice(QT * qt, QT * (qt + 1))
                    nkc_q = 4 * (qt + 1)
                    op0 = psO.tile([128, QT], F32, tag="o")
                    op1 = psO.tile([128, QT], F32, tag="o")
                    op = [op0, op1]
                    for kc in range(nkc_q):
                        pair = psP.tile([128, 2 * QT], F32, tag="p")
                        ks = slice(KCH * kc, KCH * (kc + 1))
                        diag = kc >= 4 * qt
                        nc.tensor.matmul(pair[:, 0:QT], kt_b[0:64, ks],
                                         qt_b[0:64, qs], start=True,
                                         stop=not diag, tile_position=(0, 0))
                        nc.tensor.matmul(pair[:, QT:2 * QT], kt_b[64:128, ks],
                                         qt_b[64:128, qs], start=True,
                                         stop=not diag, tile_position=(64, 0))
                        if diag:
                            r = kc - 4 * qt
                            mw = 128 * (r + 1)
                            nc.tensor.matmul(pair[:, 0:mw], id_sb[:],
                                             mb_sb[:, r, 0:mw], start=False, stop=True)
                            nc.tensor.matmul(pair[:, QT:QT + mw], id_sb[:],
                                             mb_sb[:, r, 0:mw], start=False, stop=True)
                        pt = ppool.tile([128, 2 * QT], BF16, tag="pt")
                        nc.scalar.activation(pt[:], pair[:],
                                             mybir.ActivationFunctionType.Exp,
                                             scale=0.125)
                        if debug_taps and b == 0 and qt == 0 and kc == 0:
                            nc.sync.dma_start(taps["pt0"][:], pt[:])
                        for h in range(2):
                            nc.tensor.matmul(op[h][:], vp[:, kc, h, :],
                                             pt[:, QT * h:QT * (h + 1)],
                                             start=(kc == 0), stop=(kc == nkc_q - 1))
                    for h in range(2):
                        rstg = rcpool.tile([64, QT], F32, tag="rs")
                        nc.scalar.copy(rstg[:], op[h][0:64, :])
                        ostg = rcpool.tile([64, QT], F32, tag="os")
                        nc.scalar.copy(ostg[:], op[h][64:128, :])
                        rec = rcpool.tile([64, QT], F32, tag="rc")
                        nc.vector.reciprocal_approx_fast(rec[:], rstg[:])
                        if debug_taps and b == 0 and qt == 0:
                            opd = rcpool.tile([128, QT], F32, tag="opd")
                            nc.vector.tensor_copy(opd[:], op[h][:])
                            nc.sync.dma_start(taps[f"op{h}"][:], opd[:])
                            if h == 0:
                                nc.sync.dma_start(taps["rec0"][:], rec[:])
                        nc.vector.tensor_mul(ot_b[64 * h:64 * h + 64, qs],
                                             ostg[:], rec[:])
                    # Wo partial for this qt's columns, interleaved for PE gap-fill
                    for ec in range(8):
                        yps = psO.tile([128, QT], F32, tag="o")
                        nc.tensor.matmul(yps[:], wo_sb[:, ec, :], ot_b[:, qs],
                                         start=True, stop=True)
                        ysb = ypool.tile([128, QT], BF16, tag="y")
                        nc.vector.tensor_copy(ysb[:], yps[:])
                        nc.sync.dma_start(
                            yT[128 * ec:128 * (ec + 1),
                               base + QT * qt: base + QT * (qt + 1)], ysb[:])

                if debug_taps and b == 0:
                    nc.sync.dma_start(taps["ot0"][:], ot_b[:])
            if rep_ctx is not None:
                rep_ctx.__exit__(None, None, None)
    nc.finalize()
    return nc


def host_inputs(x, token_pos, Wq, Wk, Wv, Wo, b_count=B, s_len=S):
    """Build the 8 per-core input maps (host-side prep: transpose/cast/tables)."""
    bs = b_count * s_len
    bf = ml_dtypes.bfloat16
    xTf = np.ascontiguousarray(
        np.asarray(x, np.float32).reshape(bs, D).T).astype(bf)

    tp = np.asarray(token_pos, np.float64)[:s_len]
    freqs = 1.0 / THETA ** (np.arange(0, DK, 2, dtype=np.float64) / DK)  # [32]
    ang = tp[:, None] * freqs[None, :]                  # [S, 32]
    cos = np.cos(ang).T                                 # [32, S]
    sin = np.sin(ang).T
    # d-major rows for one head (64): row d -> table d//2; sign s*(-1 if even)
    C64 = np.repeat(cos, 2, axis=0).astype(np.float32)  # [64, S]
    S64 = np.repeat(sin, 2, axis=0).astype(np.float32)
    sgn = np.where(np.arange(64) % 2 == 0, -1.0, 1.0)[:, None].astype(np.float32)
    ropeC = np.ascontiguousarray(np.tile(C64, (2, 1)))            # [128, S]
    ropeS = np.ascontiguousarray(np.tile(S64 * sgn, (2, 1)))      # [128, S]

    k_idx = np.arange(128)[:, None]
    j_idx = np.arange(QT)[None, :]
    masks = []
    for r in range(4):
        masks.append(np.where(j_idx >= 128 * r + k_idx, 0.0, MASK_NEG))
    maskb = np.concatenate(masks, axis=1).astype(bf)     # [128, 2048]
    identity = np.eye(128, dtype=np.float32).astype(bf)

    WqT = np.asarray(Wq, np.float32).T.astype(bf)        # [din, dout]
    WkT = np.asarray(Wk, np.float32).T.astype(bf)
    WvT = np.asarray(Wv, np.float32).T.astype(bf)
    WoT = np.asarray(Wo, np.float32).T.astype(bf)        # [d, e]

    in_maps = []
    for c in range(NC):
        sl = slice(DSL * c, DSL * (c + 1))
        in_maps.append({
            "xT": xTf,
            "wq": np.ascontiguousarray(WqT[:, sl]),
            "wk": np.ascontiguousarray(WkT[:, sl]),
            "wv": np.ascontiguousarray(WvT[:, sl]),
            "wo": np.ascontiguousarray(WoT[sl, :]),
            "ropeC": ropeC,
            "ropeS": ropeS,
            "maskb": maskb,
            "ident": identity,
        })
    return in_maps


def unshard(results, b_count=B, s_len=S):
    bs = b_count * s_len
    acc = np.zeros((D, bs), np.float32)
    for r in results:
        acc += np.asarray(r["yT"]).astype(np.float32)
    return np.ascontiguousarray(acc.T).reshape(b_count, s_len, D)


def kernel(x, token_pos, Wq, Wk, Wv, Wo):
    from concourse.bass_utils import run_bass_kernel_spmd
    nc = build_bass()
    in_maps = host_inputs(x, token_pos, Wq, Wk, Wv, Wo)
    res = run_bass_kernel_spmd(nc, in_maps, list(range(NC)))
    return unshard(res.results).astype(np.float32)
